# revision 23
# baseline (speedup 1.0000x reference)
"""Cross-attention kernel for TRN2, 8 NeuronCores.

Sharding: core c -> (batch b = c//2, head-group g = c%2).  Each head-group is
8 heads = 512 of the 1024 d_model channels.  All operands bf16 (halves DMA
and lets every matmul run 1 cycle/row at any free size); psum accumulation f32.

Per core (s = 512 shard channels, 4 head-pairs o):
  KT[s, lkv]   = Wk_g^T-contraction over d                    (proj, bf16)
  QT[s, lq]    = (Wq_g/8)^T q                                 (scale folded)
  Vp[lkv,h,65] = kv Wv_h + bias, 65th col = ones (denominator trick)
  ST[lkv, lq]  per head = Kh Qh^T-contraction over dh=64      (psum tile/t)
  P = exp(ST) -> bf16 SBUF                                     (Act engine)
  ctx[lq, 65]  per (head, lq-chunk) = P^T-stationary @ Vp      (free=65!)
                 col 64 = softmax denominator per q partition
  ctx_norm = ctx[:,0:64] * 1/ctx[:,64]                         (DVE)
  cT[s, lq]    = dma-transpose of ctx_norm                     (DMA xbar)
  out[lq, d]  += cT^T @ Wo_g                                   (psum over so)
Host sums the two head-group partials per batch and adds bo.

Emission is software-pipelined: projection work (K/Q/V chunks) is interleaved
as "filler" into the score/exp stream so the PE never idles while the Act
engine chews the 128-tile exp stream; attn@V waves for head-pair o run during
head-pair o+1's score loop (pt pool bufs=4 carries the P tiles across).
"""

import sys
if "/opt/trn_rl_repo" not in sys.path:
    sys.path.insert(0, "/opt/trn_rl_repo")

import ml_dtypes
import numpy as np

import concourse.bass as bass
import concourse.mybir as mybir
import concourse.tile as tile
from concourse.bass_utils import run_bass_kernel_spmd

f32 = mybir.dt.float32
bf16 = mybir.dt.bfloat16
EXP = mybir.ActivationFunctionType.Exp

D = 1024        # d_model
S = 512         # per-core channel shard (8 heads x 64)
LQ = 512
LKV = 2048
CO = D // 128   # 8 contraction chunks
SO = S // 128   # 4 shard s-tiles (head pairs)
NT = LKV // 128  # 16 lkv tiles
NKC = LKV // 512  # 4 lkv 512-chunks


def _split_multi_waits(nc, max_waits=1):
    """This container's walrus allows only `max_waits` sync-wait commands per
    instruction; hoist the excess into standalone EventSemaphore insts."""
    ev_id = 0
    for f in nc.m.functions:
        for bb in f.blocks:
            new = []
            changed = False
            for inst in bb.instructions:
                si = inst.sync_info
                if si is not None and si.on_wait and len(si.on_wait) > max_waits:
                    waits = list(si.on_wait)
                    for sw in waits[:-max_waits]:
                        ev = mybir.InstEventSemaphore(
                            name=f"EVSPLIT-{ev_id}", engine=inst.engine,
                            sync_info=mybir.SyncInfo(on_wait=[sw], on_update=[]))
                        ev_id += 1
                        nc.register_instruction(ev, overwrite=True)
                        new.append(ev)
                    inst.sync_info = mybir.SyncInfo(
                        on_wait=waits[-max_waits:], on_update=list(si.on_update))
                    changed = True
                new.append(inst)
            if changed:
                bb.instructions = new
    return nc


def _build():
    nc = bass.Bass(trn_type="TRN2")

    # DRAM I/O (pre-laid-out on host, bf16 except biases/out)
    qT = nc.dram_tensor("qT", [128, CO, LQ], bf16, kind="ExternalInput")
    kvT = nc.dram_tensor("kvT", [128, NKC, CO, 512], bf16, kind="ExternalInput")
    wqT = nc.dram_tensor("wqT", [128, SO, CO, 128], bf16, kind="ExternalInput")
    wkT = nc.dram_tensor("wkT", [128, SO, CO, 128], bf16, kind="ExternalInput")
    wvT = nc.dram_tensor("wvT", [128, CO, S], bf16, kind="ExternalInput")
    woT = nc.dram_tensor("woT", [128, SO, D], bf16, kind="ExternalInput")
    bqk = nc.dram_tensor("bqk", [128, 2 * SO], f32, kind="ExternalInput")
    bvb = nc.dram_tensor("bvb", [128, 8, 64], f32, kind="ExternalInput")
    out = nc.dram_tensor("out", [SO, 128, D], f32, kind="ExternalOutput")

    with tile.TileContext(nc) as tc:
        with tc.tile_pool(name="wgt", bufs=1) as wgt, \
             tc.tile_pool(name="big", bufs=1) as big, \
             tc.tile_pool(name="ptp", bufs=4) as ptp, \
             tc.tile_pool(name="sml", bufs=8) as sml, \
             tc.tile_pool(name="ostg", bufs=4) as ostg, \
             tc.tile_pool(name="psS", bufs=2, space="PSUM") as psS, \
             tc.tile_pool(name="psC", bufs=2, space="PSUM") as psC, \
             tc.tile_pool(name="psP", bufs=2, space="PSUM") as psP:

            # ---- resident inputs ----
            kv_sb = wgt.tile([128, NKC, CO, 512], bf16, name="kv_sb")
            wq_sb = wgt.tile([128, SO, CO, 128], bf16, name="wq_sb")
            wk_sb = wgt.tile([128, SO, CO, 128], bf16, name="wk_sb")
            wv_sb = wgt.tile([128, CO, S], bf16, name="wv_sb")
            wo_sb = wgt.tile([128, SO, D], bf16, name="wo_sb")
            qT_sb = wgt.tile([128, CO, LQ], bf16, name="qT_sb")
            bqk_sb = wgt.tile([128, 2 * SO], f32, name="bqk_sb")
            bvb_sb = wgt.tile([128, 8, 64], f32, name="bvb_sb")

            # ---- resident intermediates ----
            KT_sb = big.tile([128, SO, LKV], bf16, name="KT_sb")    # (s, lkv)
            QT_sb = big.tile([128, SO, LQ], bf16, name="QT_sb")     # (s, lq)
            # V padded per head with a ones column -> attn@V also emits the
            # softmax denominator (psum col 64 per q-partition).
            Vp_sb = big.tile([128, NT, 8, 65], bf16, name="Vp_sb")
            # ctx, normalized, [q-chunk part, pair o, q-chunk, head, dh]
            ctx_sb = big.tile([128, SO, SO, 2, 64], bf16, name="ctx_sb")
            cT_sb = big.tile([128, SO, LQ], bf16, name="cT_sb")     # (s, lq)

            # ---- DMA loads, ordered by first use; head splits so K(0,0)
            # ---- and Q(0) start as early as possible ----
            nc.sync.dma_start(bqk_sb, bqk[:])
            nc.sync.dma_start(wk_sb[:, 0], wkT[:, 0])
            nc.sync.dma_start(kv_sb[:, 0, 0:4], kvT[:, 0, 0:4])
            nc.sync.dma_start(kv_sb[:, 0, 4:8], kvT[:, 0, 4:8])
            nc.sync.dma_start(wq_sb[:, 0], wqT[:, 0])
            nc.sync.dma_start(qT_sb[:, 0:4], qT[:, 0:4])
            nc.sync.dma_start(qT_sb[:, 4:8], qT[:, 4:8])
            nc.sync.dma_start(kv_sb[:, 1], kvT[:, 1])
            nc.sync.dma_start(wk_sb[:, 1:4], wkT[:, 1:4])
            nc.sync.dma_start(bvb_sb, bvb[:])
            nc.sync.dma_start(wv_sb, wvT[:])
            nc.sync.dma_start(kv_sb[:, 2], kvT[:, 2])
            nc.sync.dma_start(wq_sb[:, 1:4], wqT[:, 1:4])
            nc.sync.dma_start(kv_sb[:, 3], kvT[:, 3])
            nc.sync.dma_start(wo_sb, woT[:])

            # ones column of Vp (denominator trick)
            nc.gpsimd.memset(Vp_sb[:, :, :, 64:65], 1.0)

            # ---- work-chunk emitters (each ~1.7us of PE) ----
            def k_chunk(o, ch):
                ps = psP.tile([128, 512], f32, name="kps", tag="proj")
                for c in range(CO):
                    nc.tensor.matmul(
                        ps, wk_sb[:, o, c, :],
                        kv_sb[:, ch, c, :], start=(c == 0), stop=(c == CO - 1))
                nc.vector.tensor_scalar_add(
                    KT_sb[:, o, ch * 512:(ch + 1) * 512], ps,
                    bqk_sb[:, SO + o:SO + o + 1])

            def q_chunk(so):
                ps = psP.tile([128, 512], f32, name="qps", tag="proj")
                for c in range(CO):
                    nc.tensor.matmul(
                        ps, wq_sb[:, so, c, :],
                        qT_sb[:, c, :], start=(c == 0), stop=(c == CO - 1))
                nc.vector.tensor_scalar_add(
                    QT_sb[:, so, :], ps, bqk_sb[:, so:so + 1])

            def v_chunk(t):
                ps = psP.tile([128, 512], f32, name="vps", tag="proj")
                ch, tt = t // 4, t % 4
                for c in range(CO):
                    nc.tensor.matmul(
                        ps, kv_sb[:, ch, c, tt * 128:(tt + 1) * 128],
                        wv_sb[:, c, :], start=(c == 0), stop=(c == CO - 1))
                nc.vector.tensor_add(
                    Vp_sb[:, t, :, 0:64],
                    ps.rearrange("p (h d) -> p h d", h=8), bvb_sb)

            # ---- attention state ----
            pt_tiles = {}   # (o, h) -> [128, NT, 512] bf16 P^T tiles
            ctx_ps = {}     # (o, h) -> [128, 4, 65] psum (4 q-chunks, 1 bank)

            def scores2(o, s):
                # two lkv tiles (t=2s, 2s+1) per head; one exp instruction
                # per head covers both tiles (1024-wide, halves Act overhead)
                stA = psS.tile([128, 2, 512], f32, name="stA", tag="sc")
                stB = psS.tile([128, 2, 512], f32, name="stB", tag="sc")
                for j in range(2):
                    t = 2 * s + j
                    nc.tensor.matmul(stA[:, j, :],
                                     KT_sb[0:64, o, t * 128:(t + 1) * 128],
                                     QT_sb[0:64, o, :], start=True, stop=True)
                    nc.tensor.matmul(stB[:, j, :],
                                     KT_sb[64:128, o, t * 128:(t + 1) * 128],
                                     QT_sb[64:128, o, :], start=True, stop=True)
                nc.scalar.activation(
                    pt_tiles[(o, 0)][:, 2 * s:2 * s + 2, :], stA, EXP)
                nc.scalar.activation(
                    pt_tiles[(o, 1)][:, 2 * s:2 * s + 2, :], stB, EXP)

            def av_phase(o, p):
                # 32 mms: lkv tiles 4p..4p+3, all 4 q-chunks, both heads.
                # All 4 q-chunk accumulators of one head share a single psum
                # bank; only the very first mm uses start=True (the psum
                # zero-region covers the whole bank), everything else
                # accumulates with start=False.
                for h in (0, 1):
                    if p == 0:
                        ctx_ps[(o, h)] = psC.tile(
                            [128, 4, 65], f32, name=f"ctx{h}", tag="ctx")
                    ctx = ctx_ps[(o, h)]
                    for qc in range(4):
                        for tp in range(4 * p, 4 * p + 4):
                            nc.tensor.matmul(
                                ctx[:, qc, :],
                                pt_tiles[(o, h)][:, tp, qc * 128:(qc + 1) * 128],
                                Vp_sb[:, tp, 2 * o + h, :],
                                start=(p == 0 and qc == 0 and tp == 0),
                                stop=(p == 3 and tp == NT - 1),
                                skip_group_check=True)

            def norm_o(o):
                # normalize all 8 q-chunk/head blocks of pair o, then
                # dma-transpose the four [q,128]x[128] blocks into cT
                for h in (0, 1):
                    ctx = ctx_ps.pop((o, h))
                    rcp = sml.tile([128, 4, 1], f32, name="rcp", tag="rcp")
                    nc.vector.reciprocal(rcp, ctx[:, :, 64:65])
                    for qc in range(4):
                        nc.vector.tensor_scalar_mul(
                            ctx_sb[:, o, qc, h, :], ctx[:, qc, 0:64],
                            rcp[:, qc, :])
                # one [128 q, 512] -> [512, 128] xbar transpose per pair:
                # out rows (qc-block j, s-in-pair p') map to cT[p', o, j*128+f]
                nc.sync.dma_start_transpose(
                    cT_sb[:, o, :].rearrange("p (j f) -> p j f", j=SO),
                    ctx_sb[:, o])

            def out_proj(lt):
                ot = ostg.tile([128, D], f32, name="ot", tag="ot")
                for dc in range(2):
                    ps = psP.tile([128, 512], f32, name="ops", tag="proj")
                    for so in range(SO):
                        nc.tensor.matmul(
                            ps, cT_sb[:, so, lt * 128:(lt + 1) * 128],
                            wo_sb[:, so, dc * 512:(dc + 1) * 512],
                            start=(so == 0), stop=(so == SO - 1))
                    if dc == 0:
                        nc.scalar.copy(ot[:, 0:512], ps)
                    else:
                        nc.vector.tensor_copy(ot[:, 512:1024], ps)
                nc.sync.dma_start(out[lt], ot)

            # ---- fillers per (o, step), ordered to match DMA arrivals and
            # ---- the just-in-time needs of scores/AV ----
            K, Q, V = k_chunk, q_chunk, v_chunk
            fillers = {
                0: {0: [lambda: K(0, 1)],
                    1: [lambda: K(1, 0)],
                    2: [lambda: K(1, 1), lambda: V(0)],
                    3: [lambda: V(1), lambda: V(2), lambda: V(3),
                        lambda: K(0, 2)],
                    4: [lambda: V(4), lambda: V(5), lambda: K(1, 2)],
                    5: [lambda: V(6), lambda: V(7), lambda: K(0, 3)],
                    6: [lambda: V(8), lambda: V(9), lambda: K(1, 3)],
                    7: [lambda: V(10), lambda: V(11), lambda: Q(1)]},
                1: {0: [lambda: V(12), lambda: V(13), lambda: V(14),
                        lambda: V(15)],
                    1: [lambda: K(2, 0)],
                    3: [lambda: K(2, 1), lambda: Q(2)],
                    5: [lambda: K(2, 2)],
                    7: [lambda: K(2, 3), lambda: Q(3)]},
                2: {1: [lambda: K(3, 0)],
                    3: [lambda: K(3, 1)],
                    5: [lambda: K(3, 2)],
                    7: [lambda: K(3, 3)]},
                3: {},
            }

            # ---- head: first K chunk + first Q chunk ----
            k_chunk(0, 0)
            q_chunk(0)

            # ---- main pipelined loops: 8 steps of 2 lkv tiles each ----
            for o in range(SO):
                pt_tiles[(o, 0)] = ptp.tile([128, NT, 512], bf16,
                                            name="ptA", tag="pt")
                pt_tiles[(o, 1)] = ptp.tile([128, NT, 512], bf16,
                                            name="ptB", tag="pt")
                fl = fillers[o]
                for s in range(8):
                    scores2(o, s)
                    for thunk in fl.get(s, ()):
                        thunk()
                    # AV trails its own exp stream; the last phase (needs
                    # exp of t=15) lands in the next loop / tail
                    if s in (3, 5, 7):
                        av_phase(o, (s - 3) // 2)
                    if o >= 1:
                        if s == 1:
                            av_phase(o - 1, 3)
                        if s == 2:
                            norm_o(o - 1)

            # ---- tail: last AV phase, normalize, transpose, out-proj ----
            av_phase(3, 3)
            norm_o(3)
            for lt in range(SO):
                out_proj(lt)

    return _split_multi_waits(nc)


_NC = None


def _get_nc():
    global _NC
    if _NC is None:
        _NC = _build()
    return _NC


def _shard(q, kv, Wq, bq, Wk, bk, Wv, bv, Wo, bo):
    bf = ml_dtypes.bfloat16

    def lay(a2d, co):  # [co*128, F] -> [128, co, F] bf16
        F = a2d.shape[1]
        return np.ascontiguousarray(
            a2d.reshape(co, 128, F).transpose(1, 0, 2)).astype(bf)

    in_maps = []
    for core in range(8):
        b, g = core // 2, core % 2
        sl = slice(g * S, (g + 1) * S)
        kvt = lay(np.ascontiguousarray(kv[b].T), CO)  # [128, 8, 2048]
        kvt = np.ascontiguousarray(
            kvt.reshape(128, CO, NKC, 512).transpose(0, 2, 1, 3))
        bqk_arr = np.concatenate([
            (bq[sl] * 0.125).reshape(SO, 128).T,
            bk[sl].reshape(SO, 128).T], axis=1)

        def lay4(w):  # [S, D] -> [128, SO, CO, 128] (per-so contiguous)
            a = lay(np.ascontiguousarray(w.T), CO)  # [128, CO, S]
            return np.ascontiguousarray(
                a.reshape(128, CO, SO, 128).transpose(0, 2, 1, 3))

        m = {
            "qT": lay(np.ascontiguousarray(q[b].T), CO),
            "kvT": kvt,
            "wqT": lay4(Wq[sl] * 0.125),
            "wkT": lay4(Wk[sl]),
            "wvT": lay(np.ascontiguousarray(Wv[sl].T), CO),
            "woT": lay(np.ascontiguousarray(Wo[:, sl].T), SO),
            "bqk": np.ascontiguousarray(bqk_arr, dtype=np.float32),
            "bvb": np.ascontiguousarray(
                np.broadcast_to(bv[sl].reshape(1, 8, 64), (128, 8, 64)),
                dtype=np.float32),
        }
        in_maps.append(m)
    return in_maps


def _run(in_maps, trace=False):
    res = run_bass_kernel_spmd(_get_nc(), in_maps, core_ids=list(range(8)),
                               trace=trace)
    return res


def kernel(q, kv, Wq, bq, Wk, bk, Wv, bv, Wo, bo, _trace=False):
    q, kv = np.asarray(q, np.float32), np.asarray(kv, np.float32)
    Wq, Wk = np.asarray(Wq, np.float32), np.asarray(Wk, np.float32)
    Wv, Wo = np.asarray(Wv, np.float32), np.asarray(Wo, np.float32)
    bq, bk = np.asarray(bq, np.float32), np.asarray(bk, np.float32)
    bv, bo = np.asarray(bv, np.float32), np.asarray(bo, np.float32)

    in_maps = _shard(q, kv, Wq, bq, Wk, bk, Wv, bv, Wo, bo)
    res = _run(in_maps, trace=_trace)
    B = q.shape[0]
    outp = np.empty((B, LQ, D), np.float32)
    for b in range(B):
        p0 = res.results[2 * b]["out"].reshape(LQ, D)
        p1 = res.results[2 * b + 1]["out"].reshape(LQ, D)
        outp[b] = p0 + p1 + bo[None, :]
    if _trace:
        kernel._last_exec_ns = res.exec_time_ns
        kernel._last_trace = res.instructions_and_trace
    return outp


# revision 25
# speedup vs baseline: 1.0370x; 1.0370x over previous
"""Cross-attention kernel for TRN2, 8 NeuronCores.

Sharding: core c -> (batch b = c//2, head-group g = c%2).  Each head-group is
8 heads = 512 of the 1024 d_model channels.  All operands bf16 (halves DMA
and lets every matmul run 1 cycle/row at any free size); psum accumulation f32.

Per core (s = 512 shard channels, 4 head-pairs o):
  KT[s, lkv]   = Wk_g^T-contraction over d                    (proj, bf16)
  QT[s, lq]    = (Wq_g/8)^T q                                 (scale folded)
  Vp[lkv,h,65] = kv Wv_h + bias, 65th col = ones (denominator trick)
  ST[lkv, lq]  per head = Kh Qh^T-contraction over dh=64      (psum tile/t)
  P = exp(ST) -> bf16 SBUF                                     (Act engine)
  ctx[lq, 65]  per (head, lq-chunk) = P^T-stationary @ Vp      (free=65!)
                 col 64 = softmax denominator per q partition
  ctx_norm = ctx[:,0:64] * 1/ctx[:,64]                         (DVE)
  cT[s, lq]    = dma-transpose of ctx_norm                     (DMA xbar)
  out[lq, d]  += cT^T @ Wo_g                                   (psum over so)
Host sums the two head-group partials per batch and adds bo.

Emission is software-pipelined: projection work (K/Q/V chunks) is interleaved
as "filler" into the score/exp stream so the PE never idles while the Act
engine chews the 128-tile exp stream; attn@V waves for head-pair o run during
head-pair o+1's score loop (pt pool bufs=4 carries the P tiles across).
"""

import sys
if "/opt/trn_rl_repo" not in sys.path:
    sys.path.insert(0, "/opt/trn_rl_repo")

import ml_dtypes
import numpy as np

import concourse.bass as bass
import concourse.mybir as mybir
import concourse.tile as tile
from concourse.bass_utils import run_bass_kernel_spmd

f32 = mybir.dt.float32
bf16 = mybir.dt.bfloat16
EXP = mybir.ActivationFunctionType.Exp

D = 1024        # d_model
S = 512         # per-core channel shard (8 heads x 64)
LQ = 512
LKV = 2048
CO = D // 128   # 8 contraction chunks
SO = S // 128   # 4 shard s-tiles (head pairs)
NT = LKV // 128  # 16 lkv tiles
NKC = LKV // 512  # 4 lkv 512-chunks


def _split_multi_waits(nc, max_waits=1):
    """This container's walrus allows only `max_waits` sync-wait commands per
    instruction; hoist the excess into standalone EventSemaphore insts."""
    ev_id = 0
    for f in nc.m.functions:
        for bb in f.blocks:
            new = []
            changed = False
            for inst in bb.instructions:
                si = inst.sync_info
                if si is not None and si.on_wait and len(si.on_wait) > max_waits:
                    waits = list(si.on_wait)
                    for sw in waits[:-max_waits]:
                        ev = mybir.InstEventSemaphore(
                            name=f"EVSPLIT-{ev_id}", engine=inst.engine,
                            sync_info=mybir.SyncInfo(on_wait=[sw], on_update=[]))
                        ev_id += 1
                        nc.register_instruction(ev, overwrite=True)
                        new.append(ev)
                    inst.sync_info = mybir.SyncInfo(
                        on_wait=waits[-max_waits:], on_update=list(si.on_update))
                    changed = True
                new.append(inst)
            if changed:
                bb.instructions = new
    return nc


def _build():
    nc = bass.Bass(trn_type="TRN2")

    # DRAM I/O (pre-laid-out on host, bf16 except biases/out)
    qT = nc.dram_tensor("qT", [128, CO, LQ], bf16, kind="ExternalInput")
    kvT = nc.dram_tensor("kvT", [128, NKC, CO, 512], bf16, kind="ExternalInput")
    wqT = nc.dram_tensor("wqT", [128, SO, CO, 128], bf16, kind="ExternalInput")
    wkT = nc.dram_tensor("wkT", [128, SO, CO, 128], bf16, kind="ExternalInput")
    wvT = nc.dram_tensor("wvT", [128, CO, S], bf16, kind="ExternalInput")
    woT = nc.dram_tensor("woT", [128, SO, D], bf16, kind="ExternalInput")
    bqk = nc.dram_tensor("bqk", [128, 2 * SO], f32, kind="ExternalInput")
    bvb = nc.dram_tensor("bvb", [128, 8, 64], f32, kind="ExternalInput")
    out = nc.dram_tensor("out", [SO, 128, D], f32, kind="ExternalOutput")

    with tile.TileContext(nc) as tc:
        with tc.tile_pool(name="wgt", bufs=1) as wgt, \
             tc.tile_pool(name="big", bufs=1) as big, \
             tc.tile_pool(name="ptp", bufs=4) as ptp, \
             tc.tile_pool(name="sml", bufs=8) as sml, \
             tc.tile_pool(name="ostg", bufs=4) as ostg, \
             tc.tile_pool(name="psS", bufs=2, space="PSUM") as psS, \
             tc.tile_pool(name="psC", bufs=2, space="PSUM") as psC, \
             tc.tile_pool(name="psP", bufs=2, space="PSUM") as psP:

            # ---- resident inputs ----
            kv_sb = wgt.tile([128, NKC, CO, 512], bf16, name="kv_sb")
            wq_sb = wgt.tile([128, SO, CO, 128], bf16, name="wq_sb")
            wk_sb = wgt.tile([128, SO, CO, 128], bf16, name="wk_sb")
            wv_sb = wgt.tile([128, CO, S], bf16, name="wv_sb")
            wo_sb = wgt.tile([128, SO, D], bf16, name="wo_sb")
            qT_sb = wgt.tile([128, CO, LQ], bf16, name="qT_sb")
            bqk_sb = wgt.tile([128, 2 * SO], f32, name="bqk_sb")
            bvb_sb = wgt.tile([128, 8, 64], f32, name="bvb_sb")

            # ---- resident intermediates ----
            KT_sb = big.tile([128, SO, LKV], bf16, name="KT_sb")    # (s, lkv)
            QT_sb = big.tile([128, SO, LQ], bf16, name="QT_sb")     # (s, lq)
            # V padded per head with a ones column -> attn@V also emits the
            # softmax denominator (psum col 64 per q-partition).
            Vp_sb = big.tile([128, NT, 8, 65], bf16, name="Vp_sb")
            # ctx, normalized, [q-chunk part, pair o, q-chunk, head, dh]
            ctx_sb = big.tile([128, SO, SO, 2, 64], bf16, name="ctx_sb")
            cT_sb = big.tile([128, SO, LQ], bf16, name="cT_sb")     # (s, lq)

            # ---- DMA loads, ordered by first use; head splits so K(0,0)
            # ---- and Q(0) start as early as possible ----
            nc.sync.dma_start(bqk_sb, bqk[:])
            nc.sync.dma_start(wk_sb[:, 0], wkT[:, 0])
            nc.sync.dma_start(kv_sb[:, 0, 0:4], kvT[:, 0, 0:4])
            nc.sync.dma_start(kv_sb[:, 0, 4:8], kvT[:, 0, 4:8])
            nc.sync.dma_start(wq_sb[:, 0], wqT[:, 0])
            nc.sync.dma_start(qT_sb[:, 0:4], qT[:, 0:4])
            nc.sync.dma_start(qT_sb[:, 4:8], qT[:, 4:8])
            nc.sync.dma_start(kv_sb[:, 1], kvT[:, 1])
            nc.sync.dma_start(wk_sb[:, 1:4], wkT[:, 1:4])
            nc.sync.dma_start(bvb_sb, bvb[:])
            nc.sync.dma_start(wv_sb, wvT[:])
            nc.sync.dma_start(kv_sb[:, 2], kvT[:, 2])
            nc.sync.dma_start(wq_sb[:, 1:4], wqT[:, 1:4])
            nc.sync.dma_start(kv_sb[:, 3], kvT[:, 3])
            nc.sync.dma_start(wo_sb, woT[:])

            # ones column of Vp (denominator trick)
            nc.gpsimd.memset(Vp_sb[:, :, :, 64:65], 1.0)

            # ---- work-chunk emitters (each ~1.7us of PE) ----
            def k_chunk(o, ch):
                ps = psP.tile([128, 512], f32, name="kps", tag="proj")
                for c in range(CO):
                    nc.tensor.matmul(
                        ps, wk_sb[:, o, c, :],
                        kv_sb[:, ch, c, :], start=(c == 0), stop=(c == CO - 1))
                nc.vector.tensor_scalar_add(
                    KT_sb[:, o, ch * 512:(ch + 1) * 512], ps,
                    bqk_sb[:, SO + o:SO + o + 1])

            def q_chunk(so):
                ps = psP.tile([128, 512], f32, name="qps", tag="proj")
                for c in range(CO):
                    nc.tensor.matmul(
                        ps, wq_sb[:, so, c, :],
                        qT_sb[:, c, :], start=(c == 0), stop=(c == CO - 1))
                nc.vector.tensor_scalar_add(
                    QT_sb[:, so, :], ps, bqk_sb[:, so:so + 1])

            def v_chunk(t):
                ps = psP.tile([128, 512], f32, name="vps", tag="proj")
                ch, tt = t // 4, t % 4
                for c in range(CO):
                    nc.tensor.matmul(
                        ps, kv_sb[:, ch, c, tt * 128:(tt + 1) * 128],
                        wv_sb[:, c, :], start=(c == 0), stop=(c == CO - 1))
                nc.vector.tensor_add(
                    Vp_sb[:, t, :, 0:64],
                    ps.rearrange("p (h d) -> p h d", h=8), bvb_sb)

            # ---- attention state ----
            pt_tiles = {}   # (o, h) -> [128, NT, 512] bf16 P^T tiles
            ctx_ps = {}     # (o, h) -> [128, 4, 65] psum (4 q-chunks, 1 bank)

            def scores2(o, s):
                # two lkv tiles (t=2s, 2s+1) per head; one exp instruction
                # per head covers both tiles (1024-wide, halves Act overhead)
                stA = psS.tile([128, 2, 512], f32, name="stA", tag="sc")
                stB = psS.tile([128, 2, 512], f32, name="stB", tag="sc")
                for j in range(2):
                    t = 2 * s + j
                    nc.tensor.matmul(stA[:, j, :],
                                     KT_sb[0:64, o, t * 128:(t + 1) * 128],
                                     QT_sb[0:64, o, :], start=True, stop=True)
                    nc.tensor.matmul(stB[:, j, :],
                                     KT_sb[64:128, o, t * 128:(t + 1) * 128],
                                     QT_sb[64:128, o, :], start=True, stop=True)
                nc.scalar.activation(
                    pt_tiles[(o, 0)][:, 2 * s:2 * s + 2, :], stA, EXP)
                nc.scalar.activation(
                    pt_tiles[(o, 1)][:, 2 * s:2 * s + 2, :], stB, EXP)

            def av_phase(o, p):
                # 32 mms: lkv tiles 4p..4p+3, all 4 q-chunks, both heads.
                # All 4 q-chunk accumulators of one head share a single psum
                # bank; only the very first mm uses start=True (the psum
                # zero-region covers the whole bank), everything else
                # accumulates with start=False.
                for h in (0, 1):
                    if p == 0:
                        ctx_ps[(o, h)] = psC.tile(
                            [128, 4, 65], f32, name=f"ctx{h}", tag="ctx")
                    ctx = ctx_ps[(o, h)]
                    for qc in range(4):
                        for tp in range(4 * p, 4 * p + 4):
                            nc.tensor.matmul(
                                ctx[:, qc, :],
                                pt_tiles[(o, h)][:, tp, qc * 128:(qc + 1) * 128],
                                Vp_sb[:, tp, 2 * o + h, :],
                                start=(p == 0 and qc == 0 and tp == 0),
                                stop=(p == 3 and tp == NT - 1),
                                skip_group_check=True)

            def norm_o(o):
                # normalize all 8 q-chunk/head blocks of pair o, then
                # dma-transpose the four [q,128]x[128] blocks into cT
                for h in (0, 1):
                    ctx = ctx_ps.pop((o, h))
                    rcp = sml.tile([128, 4, 1], f32, name="rcp", tag="rcp")
                    nc.vector.reciprocal(rcp, ctx[:, :, 64:65])
                    for qc in range(4):
                        nc.vector.tensor_scalar_mul(
                            ctx_sb[:, o, qc, h, :], ctx[:, qc, 0:64],
                            rcp[:, qc, :])
                for qc in range(4):
                    nc.sync.dma_start_transpose(
                        cT_sb[:, o, qc * 128:(qc + 1) * 128],
                        ctx_sb[:, o, qc])

            def out_proj(lt):
                ot = ostg.tile([128, D], f32, name="ot", tag="ot")
                for dc in range(2):
                    ps = psP.tile([128, 512], f32, name="ops", tag="proj")
                    for so in range(SO):
                        nc.tensor.matmul(
                            ps, cT_sb[:, so, lt * 128:(lt + 1) * 128],
                            wo_sb[:, so, dc * 512:(dc + 1) * 512],
                            start=(so == 0), stop=(so == SO - 1))
                    if dc == 0:
                        nc.scalar.copy(ot[:, 0:512], ps)
                    else:
                        nc.vector.tensor_copy(ot[:, 512:1024], ps)
                nc.sync.dma_start(out[lt], ot)

            # ---- fillers per (o, step), ordered to match DMA arrivals and
            # ---- the just-in-time needs of scores/AV ----
            K, Q, V = k_chunk, q_chunk, v_chunk
            fillers = {
                0: {0: [lambda: K(0, 1)],
                    1: [lambda: K(1, 0)],
                    2: [lambda: K(1, 1), lambda: V(0)],
                    3: [lambda: V(1), lambda: V(2), lambda: V(3),
                        lambda: K(0, 2)],
                    4: [lambda: V(4), lambda: V(5), lambda: K(1, 2)],
                    5: [lambda: V(6), lambda: V(7), lambda: K(0, 3)],
                    6: [lambda: V(8), lambda: V(9), lambda: K(1, 3)],
                    7: [lambda: V(10), lambda: V(11), lambda: Q(1)]},
                1: {0: [lambda: V(12), lambda: V(13), lambda: V(14),
                        lambda: V(15)],
                    1: [lambda: K(2, 0)],
                    3: [lambda: K(2, 1), lambda: Q(2)],
                    5: [lambda: K(2, 2)],
                    7: [lambda: K(2, 3), lambda: Q(3)]},
                2: {1: [lambda: K(3, 0)],
                    3: [lambda: K(3, 1)],
                    5: [lambda: K(3, 2)],
                    7: [lambda: K(3, 3)]},
                3: {},
            }

            # ---- head: first K chunk + first Q chunk ----
            k_chunk(0, 0)
            q_chunk(0)

            # ---- main pipelined loops: 8 steps of 2 lkv tiles each ----
            def alloc_pt(o):
                pt_tiles[(o, 0)] = ptp.tile([128, NT, 512], bf16,
                                            name="ptA", tag="pt")
                pt_tiles[(o, 1)] = ptp.tile([128, NT, 512], bf16,
                                            name="ptB", tag="pt")

            for o in range(3):
                alloc_pt(o)
                fl = fillers[o]
                for s in range(8):
                    scores2(o, s)
                    for thunk in fl.get(s, ()):
                        thunk()
                    # AV trails its own exp stream; the last phase (needs
                    # exp of t=15) lands in the next loop / tail
                    if s in (3, 5, 7):
                        av_phase(o, (s - 3) // 2)
                    if o >= 1:
                        if s == 1:
                            av_phase(o - 1, 3)
                        if s == 2:
                            norm_o(o - 1)
                    # front-run the first half of pair 3's scores inside
                    # o=2's loop so the Act stream drains earlier
                    if o == 2 and s >= 4:
                        if s == 4:
                            alloc_pt(3)
                        scores2(3, s - 4)

            # ---- pair 3 epilogue: remaining scores + trailing AV ----
            for s in range(4):
                scores2(3, s + 4)
                if s == 0:
                    av_phase(2, 3)
                if s == 1:
                    norm_o(2)
                    av_phase(3, 0)
                if s == 2:
                    av_phase(3, 1)
                if s == 3:
                    av_phase(3, 2)

            # ---- tail: last AV phase, normalize, transpose, out-proj ----
            av_phase(3, 3)
            norm_o(3)
            for lt in range(SO):
                out_proj(lt)

    return _split_multi_waits(nc)


_NC = None


def _get_nc():
    global _NC
    if _NC is None:
        _NC = _build()
    return _NC


def _shard(q, kv, Wq, bq, Wk, bk, Wv, bv, Wo, bo):
    bf = ml_dtypes.bfloat16

    def lay(a2d, co):  # [co*128, F] -> [128, co, F] bf16
        F = a2d.shape[1]
        return np.ascontiguousarray(
            a2d.reshape(co, 128, F).transpose(1, 0, 2)).astype(bf)

    in_maps = []
    for core in range(8):
        b, g = core // 2, core % 2
        sl = slice(g * S, (g + 1) * S)
        kvt = lay(np.ascontiguousarray(kv[b].T), CO)  # [128, 8, 2048]
        kvt = np.ascontiguousarray(
            kvt.reshape(128, CO, NKC, 512).transpose(0, 2, 1, 3))
        bqk_arr = np.concatenate([
            (bq[sl] * 0.125).reshape(SO, 128).T,
            bk[sl].reshape(SO, 128).T], axis=1)

        def lay4(w):  # [S, D] -> [128, SO, CO, 128] (per-so contiguous)
            a = lay(np.ascontiguousarray(w.T), CO)  # [128, CO, S]
            return np.ascontiguousarray(
                a.reshape(128, CO, SO, 128).transpose(0, 2, 1, 3))

        m = {
            "qT": lay(np.ascontiguousarray(q[b].T), CO),
            "kvT": kvt,
            "wqT": lay4(Wq[sl] * 0.125),
            "wkT": lay4(Wk[sl]),
            "wvT": lay(np.ascontiguousarray(Wv[sl].T), CO),
            "woT": lay(np.ascontiguousarray(Wo[:, sl].T), SO),
            "bqk": np.ascontiguousarray(bqk_arr, dtype=np.float32),
            "bvb": np.ascontiguousarray(
                np.broadcast_to(bv[sl].reshape(1, 8, 64), (128, 8, 64)),
                dtype=np.float32),
        }
        in_maps.append(m)
    return in_maps


def _run(in_maps, trace=False):
    res = run_bass_kernel_spmd(_get_nc(), in_maps, core_ids=list(range(8)),
                               trace=trace)
    return res


def kernel(q, kv, Wq, bq, Wk, bk, Wv, bv, Wo, bo, _trace=False):
    q, kv = np.asarray(q, np.float32), np.asarray(kv, np.float32)
    Wq, Wk = np.asarray(Wq, np.float32), np.asarray(Wk, np.float32)
    Wv, Wo = np.asarray(Wv, np.float32), np.asarray(Wo, np.float32)
    bq, bk = np.asarray(bq, np.float32), np.asarray(bk, np.float32)
    bv, bo = np.asarray(bv, np.float32), np.asarray(bo, np.float32)

    in_maps = _shard(q, kv, Wq, bq, Wk, bk, Wv, bv, Wo, bo)
    res = _run(in_maps, trace=_trace)
    B = q.shape[0]
    outp = np.empty((B, LQ, D), np.float32)
    for b in range(B):
        p0 = res.results[2 * b]["out"].reshape(LQ, D)
        p1 = res.results[2 * b + 1]["out"].reshape(LQ, D)
        outp[b] = p0 + p1 + bo[None, :]
    if _trace:
        kernel._last_exec_ns = res.exec_time_ns
        kernel._last_trace = res.instructions_and_trace
    return outp
